# revision 79
# baseline (speedup 1.0000x reference)
"""Trainium2 Bass kernel for GQA multi-head attention with RoPE.

Problem: B=2, T=2048, C=2048, 16 q-heads, 4 kv-heads, HD=128, causal, RoPE.

Sharding (8 cores): tensor-parallel over the 4 kv-head groups x data-parallel
over the 2 batch elements. Core c handles batch c//4, kv-group c%4 (4 q-heads).
Each core computes x @ wq/wk/wv for its head group, RoPE, causal attention,
and a partial output projection (rows of wo for its heads). The host sums the
4 partial outputs per batch element.

Numerics: all matmul operands are bf16 (same PE rate as float32r at free-dim
>= 256, but full rate at any width, and half the DMA/SBUF footprint); PSUM
accumulation stays fp32. Softmax skips the max-subtraction (scores are
bounded ~N(0,1) here), with the causal mask applied as a -1e5 additive bias
on diagonal blocks and fully-masked tiles skipped entirely.

Q^T/K^T layouts are produced with PE transposes of the previous chunk's rope
output (deferred one chunk so the PE stays dense), and the whole kernel lives
in one tile-pool scope with shared PSUM pools so the QKV, attention, and
output-projection phases flow into each other without barriers. Weight loads
ride the gpsimd queue (no WAR waits there), x^T tiles the sync queue, and the
previous chunk's output-projection matmuls are interleaved into the scores
stretch as exp-independent PE filler.
"""

import sys

sys.path.insert(0, "/opt/trn_rl_repo")

import ml_dtypes
import numpy as np

BF16 = ml_dtypes.bfloat16

B, T, C = 2, 2048, 2048
N_KV = 4
G = 4           # q heads per kv head
HD = 128
NCORES = 8
TT = T // 128   # 16 t-tiles
CT = C // 128   # 16 c-tiles
NTC = 4         # 512-wide t chunks
SCALE = float(1.0 / np.sqrt(HD))
MASK_BIAS = -1.0e5

_CACHE = {}
LAST_RESULTS = None

def _build():
    import concourse.bass as bass
    import concourse.tile as tile
    from concourse import mybir, bacc

    def bcast_mid(ap2d, reps):
        """[128, N] AP -> [128, reps, N] with a stride-0 middle dim."""
        return bass.AP(tensor=ap2d.tensor, offset=ap2d.offset,
                       ap=[list(ap2d.ap[0]), [0, reps], list(ap2d.ap[1])])

    f32, bf16 = mybir.dt.float32, mybir.dt.bfloat16
    fp8 = mybir.dt.float8e4

    nc = bacc.Bacc()
    xT = nc.dram_tensor("xT", [128, 8 * 4 * 4 * 256], bf16, kind="ExternalInput")
    wqkv = nc.dram_tensor("wqkv", [128, CT * 768], bf16, kind="ExternalInput")
    wo = nc.dram_tensor("wo", [128, G * C], bf16, kind="ExternalInput")
    fcos = nc.dram_tensor("fcos", [128, TT * 64], bf16, kind="ExternalInput")
    fsin = nc.dram_tensor("fsin", [128, TT * 64], bf16, kind="ExternalInput")
    cident = nc.dram_tensor("cident", [128, 128], bf16, kind="ExternalInput")
    cones = nc.dram_tensor("cones", [128, 1], bf16, kind="ExternalInput")
    cones8 = nc.dram_tensor("cones8", [128, 128], fp8, kind="ExternalInput")
    ctri = nc.dram_tensor("ctri", [128, 128], f32, kind="ExternalInput")
    out = nc.dram_tensor("out", [T, C], bf16, kind="ExternalOutput")

    from contextlib import ExitStack

    with tile.TileContext(nc) as tc:
        with ExitStack() as stack:
            ep = stack.enter_context
            cpool = ep(tc.tile_pool(name="consts", bufs=1))
            ppool = ep(tc.tile_pool(name="persist", bufs=1))
            wpool = ep(tc.tile_pool(name="weights", bufs=1))
            wop = ep(tc.tile_pool(name="wop", bufs=1))
            fpool = ep(tc.tile_pool(name="freqs", bufs=1))
            xtp = ep(tc.tile_pool(name="xt", bufs=6))
            qbfp = ep(tc.tile_pool(name="qkvbf", bufs=3))
            rtp = ep(tc.tile_pool(name="ropet", bufs=8))
            qrp = ep(tc.tile_pool(name="qr", bufs=4))
            krp = ep(tc.tile_pool(name="kr", bufs=4))
            expp = ep(tc.tile_pool(name="expt", bufs=3))
            e8p = ep(tc.tile_pool(name="e8", bufs=3))
            denp = ep(tc.tile_pool(name="denb", bufs=2))
            bcp = ep(tc.tile_pool(name="bcb", bufs=2))
            opool = ep(tc.tile_pool(name="outbuf", bufs=4))
            pa = ep(tc.tile_pool(name="pa", bufs=4, space="PSUM"))
            pb = ep(tc.tile_pool(name="pb", bufs=2, space="PSUM"))
            pc = ep(tc.tile_pool(name="pc", bufs=2, space="PSUM"))
            ident_sb = cpool.tile([128, 128], bf16)
            ones_sb = cpool.tile([128, 1], bf16)
            ones8_sb = cpool.tile([128, 128], fp8)
            tri_sb = cpool.tile([128, 128], f32)
            warm_sb = cpool.tile([128, 512], bf16)

            # ---- persistent activations ----
            qT_sb = ppool.tile([128, G, T], bf16)      # [d, h, t]
            kT_sb = ppool.tile([128, T], bf16)         # [d, s]
            v_sb = ppool.tile([128, TT, HD], bf16)     # [s%128, s//128, d]
            outT_sb = ppool.tile([128, G, T], bf16)    # [d, h, t]

            fcos_sb = fpool.tile([128, TT, 64], bf16)
            fsin_sb = fpool.tile([128, TT, 64], bf16)
            wqkv_sb = wpool.tile([128, CT, 768], bf16)
            wo_sb = wop.tile([128, G, C], bf16)
            wqkv_flat = wqkv_sb[:].rearrange("p a b -> p (a b)")

            def load_wg(wg):
                # weights ride gpsimd exclusively: that queue has no WAR
                # waits, so it drains strictly in order and can't be held
                # up behind an xt load that waits on psum readers
                nc.gpsimd.dma_start(
                    wqkv_flat[:, wg * 2 * 768:(wg + 1) * 2 * 768],
                    wqkv[:, wg * 2 * 768:(wg + 1) * 2 * 768],
                )
            def load_consts():
                # queued on gpsimd behind the 8 weight chunks; none of these
                # are needed before ~20us (first transpose / first rope batch
                # uses fcos which lands by ~12us; wo not until ~150us)
                nc.gpsimd.dma_start(ident_sb[:], cident[:])
                nc.gpsimd.dma_start(fcos_sb[:].rearrange("p a b -> p (a b)"),
                                    fcos[:])
                nc.gpsimd.dma_start(fsin_sb[:].rearrange("p a b -> p (a b)"),
                                    fsin[:])
                nc.gpsimd.dma_start(ones_sb[:], cones[:])
                nc.gpsimd.dma_start(ones8_sb[:], cones8[:])
                nc.gpsimd.dma_start(tri_sb[:], ctri[:])
                nc.gpsimd.dma_start(wo_sb[:].rearrange("p a b -> p (a b)"),
                                    wo[:])

            # PE warm-up spin during startup DMAs (HAM needs ~3.4us busy).
            # memset source: no DMA dependency, PE ramps immediately.
            nc.vector.memset(warm_sb[:], 0.0)
            warm_ps = pa.tile([128, 512], f32, tag="pa", name="warm_ps")
            for _ in range(8):
                nc.tensor.matmul(warm_ps[:], warm_sb[:, 0:128], warm_sb[:],
                                 start=True, stop=True)

            # ================= Phase 1: QKV projection + RoPE + transpose ====
            def emit_tr(tt, qr, kr):
                """PE-transpose one chunk's rope output into qT_sb/kT_sb."""
                for h in range(G):
                    ptr = pc.tile([128, 128], bf16, tag="pc", name="ptr")
                    nc.tensor.transpose(
                        ptr[:], qr[:, h * 128:(h + 1) * 128], ident_sb[:]
                    )
                    nc.scalar.copy(
                        qT_sb[:, h, tt * 128:(tt + 1) * 128], ptr[:]
                    )
                ptr = pc.tile([128, 128], bf16, tag="pc", name="ptr")
                nc.tensor.transpose(ptr[:], kr[:], ident_sb[:])
                nc.scalar.copy(kT_sb[:, tt * 128:(tt + 1) * 128], ptr[:])

            pending_tr = []
            for ch in range(T // 256):  # 8 chunks of 256 t
                scope = nc.named_scope(f"p1_ch{ch}")
                scope.__enter__()
                psq = [pa.tile([128, 512], f32, tag="pa", name="psq")
                       for _ in range(2)]
                pskv = [pb.tile([128, 256], f32, tag="pb", name="pskv")
                        for _ in range(2)]
                xts = []
                for cg in range(CT // 8):
                    if ch == 0 and cg == 0:
                        load_wg(0)
                    xt = xtp.tile([128, 8, 256], bf16, tag="xt")
                    xts.append(xt)
                    col0 = (ch * 2 + cg) * 2048
                    nc.sync.dma_start(
                        xt[:].rearrange("p a b -> p (a b)"),
                        xT[:, col0:col0 + 2048],
                    )
                for ct in range(CT):
                    # Emit each wqkv chunk load right before the first matmul
                    # that reads it (keeps reader semaphores tight)
                    if ch == 0 and ct % 2 == 0 and ct > 0:
                        load_wg(ct // 2)
                    for t2 in range(2):
                        lhsT = xts[ct // 8][:, ct % 8, t2 * 128:(t2 + 1) * 128]
                        # 4-ct sub-groups (start only on the very first, stop
                        # on each sub-group tail): sub-groups accumulate onto
                        # the same psum but keep dependency-coalescing scoped
                        # so the chain can start before late weight chunks land
                        nc.tensor.matmul(
                            psq[t2][:], lhsT, wqkv_sb[:, ct, 0:512],
                            start=(ct == 0), stop=(ct % 4 == 3),
                            skip_group_check=True,
                        )
                        nc.tensor.matmul(
                            pskv[t2][:], lhsT, wqkv_sb[:, ct, 512:768],
                            start=(ct == 0), stop=(ct % 4 == 3),
                            skip_group_check=True,
                        )
                if ch == 0:
                    load_consts()
                # downcast psum -> bf16 on Act. Emission order matters: the
                # k/v copies free the pskv psum bufs the next chunk's matmuls
                # need, so they go first; the deferred transposes' psum->qT/kT
                # copies queue behind them on Act (no urgency — their readers
                # are the attention phase).
                qkv_bfs = []
                for t2 in range(2):
                    tt = ch * 2 + t2
                    qkv_bf = qbfp.tile([128, 640], bf16, tag="qbf")
                    qkv_bfs.append(qkv_bf)
                    nc.scalar.copy(qkv_bf[:, 512:640], pskv[t2][:, 0:128])
                    nc.scalar.copy(v_sb[:, tt, :], pskv[t2][:, 128:256])
                for t2 in range(2):
                    nc.scalar.copy(qkv_bfs[t2][:, 0:512], psq[t2][:])
                # PE-transpose the PREVIOUS chunk's rope output while this
                # chunk's rope runs on DVE (keeps the PE dense)
                for args in pending_tr:
                    emit_tr(*args)
                pending_tr = []
                # rope in bf16 (DVE at 2x)
                for t2 in range(2):
                    tt = ch * 2 + t2
                    qkv_bf = qkv_bfs[t2]
                    qr = qrp.tile([128, 512], bf16, tag="qr")
                    kr = krp.tile([128, 128], bf16, tag="kr")
                    cosb = bcast_mid(fcos_sb[:, tt, :], 4)
                    sinb = bcast_mid(fsin_sb[:, tt, :], 4)
                    qsrc = qkv_bf[:, 0:512].rearrange(
                        "p (h two j) -> p h two j", h=4, two=2
                    )
                    qdst = qr[:].rearrange(
                        "p (h two j) -> p h two j", h=4, two=2
                    )
                    te4, to4 = qsrc[:, :, 0, :], qsrc[:, :, 1, :]
                    a1 = rtp.tile([128, 4, 64], bf16, tag="rt")
                    a2 = rtp.tile([128, 4, 64], bf16, tag="rt")
                    nc.vector.tensor_mul(a1[:], te4, cosb)
                    nc.vector.tensor_mul(a2[:], to4, sinb)
                    nc.vector.tensor_sub(qdst[:, :, 0, :], a1[:], a2[:])
                    b1 = rtp.tile([128, 4, 64], bf16, tag="rt")
                    b2 = rtp.tile([128, 4, 64], bf16, tag="rt")
                    nc.vector.tensor_mul(b1[:], te4, sinb)
                    nc.vector.tensor_mul(b2[:], to4, cosb)
                    nc.vector.tensor_add(qdst[:, :, 1, :], b1[:], b2[:])
                    # K rope
                    kte, kto = qkv_bf[:, 512:576], qkv_bf[:, 576:640]
                    cos1 = fcos_sb[:, tt, :]
                    sin1 = fsin_sb[:, tt, :]
                    c1 = rtp.tile([128, 64], bf16, tag="rtk")
                    c2 = rtp.tile([128, 64], bf16, tag="rtk")
                    nc.vector.tensor_mul(c1[:], kte, cos1)
                    nc.vector.tensor_mul(c2[:], kto, sin1)
                    nc.vector.tensor_sub(kr[:, 0:64], c1[:], c2[:])
                    d1 = rtp.tile([128, 64], bf16, tag="rtk")
                    d2 = rtp.tile([128, 64], bf16, tag="rtk")
                    nc.vector.tensor_mul(d1[:], kte, sin1)
                    nc.vector.tensor_mul(d2[:], kto, cos1)
                    nc.vector.tensor_add(kr[:, 64:128], d1[:], d2[:])
                    pending_tr.append((tt, qr, kr))
                scope.__exit__(None, None, None)
            for args in pending_tr:
                emit_tr(*args)

            # ================= Phase 2+3: attention + output projection ======
            # Per t-chunk: scores for a head, then den/AV. exp on Act (544ns
            # per tile) is ~2.3x slower than a scores matmul, so the PE would
            # outrun exp and stall on the psum rotation. The previous chunk's
            # WO matmuls are exp-independent: they're interleaved into the
            # sc0/sc1 stretch as PE filler while Act drains the exp backlog.
            # All psum->sbuf copies in this phase go on DVE to keep Act free
            # for exp.
            def make_wo_pieces(tc_i):
                pieces = []
                for t2 in range(4):
                    gt = tc_i * 4 + t2
                    for cc2 in range(2):
                        def piece(gt=gt, cc2=cc2, k=t2 * 2 + cc2):
                            osb = opool.tile([128, 1024], bf16, tag="osb",
                                             name="osb")
                            for half in range(2):
                                cc = cc2 * 2 + half
                                psw = pc.tile([128, 512], f32, tag="pc",
                                              name="psw")
                                for h in range(G):
                                    nc.tensor.matmul(
                                        psw[:],
                                        outT_sb[:, h,
                                                gt * 128:(gt + 1) * 128],
                                        wo_sb[:, h,
                                              cc * 512:(cc + 1) * 512],
                                        start=(h == 0), stop=(h == G - 1),
                                    )
                                nc.vector.tensor_copy(
                                    osb[:, half * 512:(half + 1) * 512],
                                    psw[:])
                            store_eng = nc.sync if k % 2 else nc.gpsimd
                            store_eng.dma_start(
                                out[gt * 128:(gt + 1) * 128,
                                    cc2 * 1024:(cc2 + 1) * 1024],
                                osb[:],
                            )
                        pieces.append(piece)
                return pieces

            pending_wo = []
            for tc_i in range(NTC):
                scope = nc.named_scope(f"attn_tc{tc_i}")
                scope.__enter__()
                t0 = tc_i * 512
                n_s = 4 * (tc_i + 1)
                offs = [128 * (si - 4 * tc_i) if si >= 4 * tc_i else 0
                        for si in range(n_s)]
                # natural si order: si=0 always has off=0 (so the first
                # matmul of each psum accumulation group resets the full
                # 512-col range), and the diagonal tiles — whose exps are
                # emitted last — land at the END of the den/AV chains,
                # giving the Act exp pipeline maximum slack
                order = list(range(n_s))
                expTs = {}

                def emit_scores(h, sis, tc_i=tc_i, t0=t0, offs=offs,
                                expTs=expTs):
                    if h not in expTs:
                        expT_t = expp.tile([128, TT, 512], bf16,
                                           tag="expT", name="expT")
                        e8_t = e8p.tile([128, 6, 2, 512], fp8, tag="e8",
                                        name="e8_t")
                        expTs[h] = (expT_t, e8_t)
                    expT, e8 = expTs[h]
                    for si in sis:
                        off = offs[si]
                        ps = pa.tile([128, 512], f32, tag="pa", name="ps")
                        nc.tensor.matmul(
                            ps[:, off:512],
                            kT_sb[:, si * 128:(si + 1) * 128],
                            qT_sb[:, h, t0 + off:t0 + 512],
                            start=True, stop=True,
                        )
                        if si >= 4 * tc_i:
                            nc.vector.tensor_add(
                                ps[:, off:off + 128],
                                ps[:, off:off + 128], tri_sb[:],
                            )
                        nc.scalar.activation(
                            expT[:, si, off:512], ps[:, off:512],
                            mybir.ActivationFunctionType.Exp, scale=SCALE,
                        )
                        if si < 4 * tc_i:
                            # fp8 copy feeding the DoubleRow den pass; on
                            # gpsimd (idle) so it never queues behind the
                            # DVE's output copies and stalls the den chain
                            nc.gpsimd.tensor_copy(
                                e8[:, si // 2, si % 2, :],
                                expT[:, si, 0:512],
                            )

                def emit_da(h, tc_i=tc_i, t0=t0, n_s=n_s, offs=offs,
                            order=order, expTs=expTs):
                    expT, e8 = expTs.pop(h)
                    # full (off-diagonal) s-tile pairs via fp8 DoubleRow
                    # (K=256 per instruction, half the PE cost); row 0 of the
                    # 64-row output carries the sum, rows 1-63 are padding to
                    # satisfy the dual-fp8 weight-load width restriction.
                    # Diagonal tiles accumulate onto row 0 in bf16.
                    psd = pb.tile([64, 512], f32, tag="pb", name="psd")
                    npair = 2 * tc_i
                    nden = npair + (n_s - 4 * tc_i)
                    for i in range(npair):
                        nc.tensor.matmul(
                            psd[:, 0:512],
                            ones8_sb[:].rearrange("p (two m) -> p two m",
                                                  two=2),
                            e8[:, i, :, :],
                            perf_mode=mybir.MatmulPerfMode.DoubleRow,
                            start=(i == 0), stop=(i == nden - 1),
                        )
                    for j, si in enumerate(range(4 * tc_i, n_s)):
                        i = npair + j
                        off = offs[si]
                        nc.tensor.matmul(
                            psd[0:1, off:512], ones_sb[:],
                            expT[:, si, off:512],
                            start=(i == 0), stop=(i == nden - 1),
                        )
                    den_r = denp.tile([1, 512], f32, tag="denr",
                                      name="den_r")
                    nc.vector.reciprocal_approx_fast(den_r[:], psd[0:1, :])
                    bc = bcp.tile([128, 512], f32, tag="bc", name="bc")
                    nc.gpsimd.partition_broadcast(bc[:], den_r[:])
                    pso = pa.tile([128, 512], f32, tag="pa", name="pso")
                    for i, si in enumerate(order):
                        off = offs[si]
                        nc.tensor.matmul(
                            pso[:, off:512], v_sb[:, si, :],
                            expT[:, si, off:512],
                            start=(i == 0), stop=(i == n_s - 1),
                        )
                    nc.vector.tensor_mul(
                        outT_sb[:, h, t0:t0 + 512], pso[:], bc[:]
                    )

                # sc0/sc1 in si-quads with the previous chunk's WO pieces as
                # filler, then the 1-deep head pipeline for the rest
                quads = [(h, list(range(q, min(q + 4, n_s))))
                         for h in (0, 1) for q in range(0, n_s, 4)]
                npw = max(0, len(pending_wo) - 2)  # save 2 for da0/da1 slots
                k = 0
                for qi, (h, sis) in enumerate(quads):
                    emit_scores(h, sis)
                    owed = ((qi + 1) * npw) // len(quads)
                    while k < owed:
                        pending_wo[k]()
                        k += 1
                while k < npw:
                    pending_wo[k]()
                    k += 1
                emit_da(0)
                if k < len(pending_wo):
                    pending_wo[k]()
                    k += 1
                emit_scores(2, range(n_s))
                emit_da(1)
                if k < len(pending_wo):
                    pending_wo[k]()
                    k += 1
                emit_scores(3, range(n_s))
                emit_da(2)
                emit_da(3)
                pending_wo = make_wo_pieces(tc_i)
                scope.__exit__(None, None, None)
            scope = nc.named_scope("wo_tc3")
            scope.__enter__()
            for p in pending_wo:
                p()
            scope.__exit__(None, None, None)

    nc.finalize()
    return nc


def _prep_host(x, freqs_cos, freqs_sin, wq, wk, wv, wo):
    """Build per-core input maps."""
    x = np.asarray(x, dtype=np.float32)
    freqs_cos = np.asarray(freqs_cos, dtype=np.float32)
    freqs_sin = np.asarray(freqs_sin, dtype=np.float32)
    wq = np.asarray(wq, dtype=np.float32)
    wk = np.asarray(wk, dtype=np.float32)
    wv = np.asarray(wv, dtype=np.float32)
    wo = np.asarray(wo, dtype=np.float32)

    perm = np.concatenate([np.arange(0, HD, 2), np.arange(1, HD, 2)])
    # xT pre-tiled: [p, ch, cg, ci, t'] so each (ch, cg) load is contiguous
    xTs = []
    for b in range(B):
        A = np.ascontiguousarray(x[b].T)           # [C, T]
        A = A.reshape(4, 4, 128, 8, 256)           # [cg, ci, p, ch, t']
        A = A.transpose(2, 3, 0, 1, 4)             # [p, ch, cg, ci, t']
        xTs.append(np.ascontiguousarray(A.reshape(128, -1).astype(BF16)))
    cident = np.eye(128, dtype=BF16)
    cones = np.ones((128, 1), dtype=BF16)
    cones8 = np.ones((128, 128), dtype=ml_dtypes.float8_e4m3)
    ds, dt = np.meshgrid(np.arange(128), np.arange(128), indexing="ij")
    ctri = np.where(ds <= dt, 0.0, MASK_BIAS).astype(np.float32)

    in_maps = []
    for c in range(NCORES):
        b, kv = c // 4, c % 4
        cols = []
        for g in range(G):
            h = kv * G + g
            cols.append(wq[:, h * HD:(h + 1) * HD][:, perm])
        cols.append(wk[:, kv * HD:(kv + 1) * HD][:, perm])
        cols.append(wv[:, kv * HD:(kv + 1) * HD])
        wqkv_c = np.concatenate(cols, axis=1)              # [C, 768]
        wqkv_c = wqkv_c.reshape(CT, 128, 768).transpose(1, 0, 2)
        wqkv_c = np.ascontiguousarray(wqkv_c.reshape(128, -1).astype(BF16))
        wo_c = wo[kv * G * HD:(kv + 1) * G * HD, :]        # [512, C]
        wo_c = wo_c.reshape(G, 128, C).transpose(1, 0, 2)
        wo_c = np.ascontiguousarray(wo_c.reshape(128, -1).astype(BF16))
        fc = np.ascontiguousarray(
            freqs_cos.reshape(TT, 128, 64).transpose(1, 0, 2)
            .reshape(128, -1).astype(BF16))
        fs = np.ascontiguousarray(
            freqs_sin.reshape(TT, 128, 64).transpose(1, 0, 2)
            .reshape(128, -1).astype(BF16))
        in_maps.append({
            "xT": xTs[b],
            "wqkv": wqkv_c,
            "wo": wo_c,
            "fcos": fc,
            "fsin": fs,
            "cident": cident,
            "cones": cones,
            "cones8": cones8,
            "ctri": ctri,
        })
    return in_maps


def _install_ntff_hook_shim():
    """bass_utils trace=True needs antenv.axon_hooks, absent in this image.
    Provide it in sys.modules and register the ctypes NTFF hook."""
    import types

    if "antenv.axon_hooks" in sys.modules:
        return
    mod = types.ModuleType("antenv.axon_hooks")
    mod._hook = None
    mod.set_axon_ntff_profile_hook = lambda h: setattr(mod, "_hook", h)
    mod.get_axon_ntff_profile_hook = lambda: mod._hook
    sys.modules["antenv.axon_hooks"] = mod
    try:
        from trn_agent_boot.trn_boot import _ntff_profile_via_ctypes

        mod._hook = _ntff_profile_via_ctypes("/opt/axon/libaxon_pjrt.so")
    except Exception:
        pass


def kernel(x, freqs_cos, freqs_sin, wq, wk, wv, wo, trace=False):
    global LAST_RESULTS
    from concourse.bass_utils import run_bass_kernel_spmd

    if trace:
        _install_ntff_hook_shim()

    if "nc" not in _CACHE:
        _CACHE["nc"] = _build()
    nc = _CACHE["nc"]

    in_maps = _prep_host(x, freqs_cos, freqs_sin, wq, wk, wv, wo)
    res = run_bass_kernel_spmd(nc, in_maps, core_ids=list(range(NCORES)),
                               trace=trace)
    LAST_RESULTS = res
    out = np.zeros((B, T, C), dtype=np.float32)
    for c in range(NCORES):
        out[c // 4] += res.results[c]["out"].astype(np.float32)
    return out


# revision 81
# speedup vs baseline: 1.3311x; 1.3311x over previous
"""Trainium2 Bass kernel for GQA multi-head attention with RoPE.

Problem: B=2, T=2048, C=2048, 16 q-heads, 4 kv-heads, HD=128, causal, RoPE.

Sharding (8 cores): tensor-parallel over the 4 kv-head groups x data-parallel
over the 2 batch elements. Core c handles batch c//4, kv-group c%4 (4 q-heads).
Each core computes x @ wq/wk/wv for its head group, RoPE, causal attention,
and a partial output projection (rows of wo for its heads). The host sums the
4 partial outputs per batch element.

Numerics: all matmul operands are bf16 (same PE rate as float32r at free-dim
>= 256, but full rate at any width, and half the DMA/SBUF footprint); PSUM
accumulation stays fp32. Softmax skips the max-subtraction (scores are
bounded ~N(0,1) here), with the causal mask applied as a -1e5 additive bias
on diagonal blocks and fully-masked tiles skipped entirely.

Q^T/K^T layouts are produced with PE transposes of the previous chunk's rope
output (deferred one chunk so the PE stays dense), and the whole kernel lives
in one tile-pool scope with shared PSUM pools so the QKV, attention, and
output-projection phases flow into each other without barriers. Weight loads
ride the gpsimd queue (no WAR waits there), x^T tiles the sync queue, and the
previous chunk's output-projection matmuls are interleaved into the scores
stretch as exp-independent PE filler.
"""

import sys

sys.path.insert(0, "/opt/trn_rl_repo")

import ml_dtypes
import numpy as np

BF16 = ml_dtypes.bfloat16

B, T, C = 2, 2048, 2048
N_KV = 4
G = 4           # q heads per kv head
HD = 128
NCORES = 8
TT = T // 128   # 16 t-tiles
CT = C // 128   # 16 c-tiles
NTC = 4         # 512-wide t chunks
SCALE = float(1.0 / np.sqrt(HD))
MASK_BIAS = -1.0e5

_CACHE = {}
LAST_RESULTS = None

def _build():
    import concourse.bass as bass
    import concourse.tile as tile
    from concourse import mybir, bacc

    def bcast_mid(ap2d, reps):
        """[128, N] AP -> [128, reps, N] with a stride-0 middle dim."""
        return bass.AP(tensor=ap2d.tensor, offset=ap2d.offset,
                       ap=[list(ap2d.ap[0]), [0, reps], list(ap2d.ap[1])])

    f32, bf16 = mybir.dt.float32, mybir.dt.bfloat16
    fp8 = mybir.dt.float8e4

    nc = bacc.Bacc()
    xT = nc.dram_tensor("xT", [128, 8 * 4 * 4 * 256], bf16, kind="ExternalInput")
    wqkv = nc.dram_tensor("wqkv", [128, CT * 768], bf16, kind="ExternalInput")
    wo = nc.dram_tensor("wo", [128, G * C], bf16, kind="ExternalInput")
    fcos = nc.dram_tensor("fcos", [128, TT * 64], bf16, kind="ExternalInput")
    fsin = nc.dram_tensor("fsin", [128, TT * 64], bf16, kind="ExternalInput")
    cident = nc.dram_tensor("cident", [128, 128], bf16, kind="ExternalInput")
    cones = nc.dram_tensor("cones", [128, 1], bf16, kind="ExternalInput")
    cones8 = nc.dram_tensor("cones8", [128, 128], fp8, kind="ExternalInput")
    ctri = nc.dram_tensor("ctri", [128, 128], f32, kind="ExternalInput")
    out = nc.dram_tensor("out", [T, C], bf16, kind="ExternalOutput")

    from contextlib import ExitStack

    with tile.TileContext(nc) as tc:
        with ExitStack() as stack:
            ep = stack.enter_context
            cpool = ep(tc.tile_pool(name="consts", bufs=1))
            ppool = ep(tc.tile_pool(name="persist", bufs=1))
            wpool = ep(tc.tile_pool(name="weights", bufs=1))
            wop = ep(tc.tile_pool(name="wop", bufs=1))
            fpool = ep(tc.tile_pool(name="freqs", bufs=1))
            xtp = ep(tc.tile_pool(name="xt", bufs=6))
            qbfp = ep(tc.tile_pool(name="qkvbf", bufs=3))
            rtp = ep(tc.tile_pool(name="ropet", bufs=8))
            qrp = ep(tc.tile_pool(name="qr", bufs=4))
            krp = ep(tc.tile_pool(name="kr", bufs=4))
            expp = ep(tc.tile_pool(name="expt", bufs=3))
            e8p = ep(tc.tile_pool(name="e8", bufs=3))
            denp = ep(tc.tile_pool(name="denb", bufs=2))
            bcp = ep(tc.tile_pool(name="bcb", bufs=2))
            opool = ep(tc.tile_pool(name="outbuf", bufs=4))
            pa = ep(tc.tile_pool(name="pa", bufs=4, space="PSUM"))
            pb = ep(tc.tile_pool(name="pb", bufs=2, space="PSUM"))
            pc = ep(tc.tile_pool(name="pc", bufs=2, space="PSUM"))
            ident_sb = cpool.tile([128, 128], bf16)
            ones_sb = cpool.tile([128, 1], bf16)
            ones8_sb = cpool.tile([128, 128], fp8)
            tri_sb = cpool.tile([128, 128], f32)
            warm_sb = cpool.tile([128, 512], bf16)

            # ---- persistent activations ----
            qT_sb = ppool.tile([128, G, T], bf16)      # [d, h, t]
            kT_sb = ppool.tile([128, T], bf16)         # [d, s]
            v_sb = ppool.tile([128, TT, HD], bf16)     # [s%128, s//128, d]
            outT_sb = ppool.tile([128, G, T], bf16)    # [d, h, t]

            fcos_sb = fpool.tile([128, TT, 64], bf16)
            fsin_sb = fpool.tile([128, TT, 64], bf16)
            wqkv_sb = wpool.tile([128, CT, 768], bf16)
            wo_sb = wop.tile([128, G, C], bf16)
            wqkv_flat = wqkv_sb[:].rearrange("p a b -> p (a b)")

            def load_wg(wg):
                # weights ride gpsimd exclusively: that queue has no WAR
                # waits, so it drains strictly in order and can't be held
                # up behind an xt load that waits on psum readers
                nc.gpsimd.dma_start(
                    wqkv_flat[:, wg * 2 * 768:(wg + 1) * 2 * 768],
                    wqkv[:, wg * 2 * 768:(wg + 1) * 2 * 768],
                )
            def load_consts():
                # queued on gpsimd behind the 8 weight chunks; none of these
                # are needed before ~20us (first transpose / first rope batch
                # uses fcos which lands by ~12us; wo not until ~150us)
                nc.gpsimd.dma_start(ident_sb[:], cident[:])
                nc.gpsimd.dma_start(fcos_sb[:].rearrange("p a b -> p (a b)"),
                                    fcos[:])
                nc.gpsimd.dma_start(fsin_sb[:].rearrange("p a b -> p (a b)"),
                                    fsin[:])
                nc.gpsimd.dma_start(ones_sb[:], cones[:])
                nc.gpsimd.dma_start(ones8_sb[:], cones8[:])
                nc.gpsimd.dma_start(tri_sb[:], ctri[:])
                nc.gpsimd.dma_start(wo_sb[:].rearrange("p a b -> p (a b)"),
                                    wo[:])

            # PE warm-up spin during startup DMAs (HAM needs ~3.4us busy).
            # memset source: no DMA dependency, PE ramps immediately.
            nc.vector.memset(warm_sb[:], 0.0)
            warm_ps = pa.tile([128, 512], f32, tag="pa", name="warm_ps")
            for _ in range(8):
                nc.tensor.matmul(warm_ps[:], warm_sb[:, 0:128], warm_sb[:],
                                 start=True, stop=True)

            # ================= Phase 1: QKV projection + RoPE + transpose ====
            def emit_tr(tt, qr, kr):
                """PE-transpose one chunk's rope output into qT_sb/kT_sb."""
                for h in range(G):
                    ptr = pc.tile([128, 128], bf16, tag="pc", name="ptr")
                    nc.tensor.transpose(
                        ptr[:], qr[:, h * 128:(h + 1) * 128], ident_sb[:]
                    )
                    nc.scalar.copy(
                        qT_sb[:, h, tt * 128:(tt + 1) * 128], ptr[:]
                    )
                ptr = pc.tile([128, 128], bf16, tag="pc", name="ptr")
                nc.tensor.transpose(ptr[:], kr[:], ident_sb[:])
                nc.scalar.copy(kT_sb[:, tt * 128:(tt + 1) * 128], ptr[:])

            pending_tr = []
            for ch in range(T // 256):  # 8 chunks of 256 t
                scope = nc.named_scope(f"p1_ch{ch}")
                scope.__enter__()
                psq = [pa.tile([128, 512], f32, tag="pa", name="psq")
                       for _ in range(2)]
                pskv = [pb.tile([128, 256], f32, tag="pb", name="pskv")
                        for _ in range(2)]
                xts = []
                for cg in range(CT // 8):
                    if ch == 0 and cg == 0:
                        load_wg(0)
                    xt = xtp.tile([128, 8, 256], bf16, tag="xt")
                    xts.append(xt)
                    col0 = (ch * 2 + cg) * 2048
                    nc.sync.dma_start(
                        xt[:].rearrange("p a b -> p (a b)"),
                        xT[:, col0:col0 + 2048],
                    )
                for ct in range(CT):
                    # Emit each wqkv chunk load right before the first matmul
                    # that reads it (keeps reader semaphores tight)
                    if ch == 0 and ct % 2 == 0 and ct > 0:
                        load_wg(ct // 2)
                    for t2 in range(2):
                        lhsT = xts[ct // 8][:, ct % 8, t2 * 128:(t2 + 1) * 128]
                        # 4-ct sub-groups (start only on the very first, stop
                        # on each sub-group tail): sub-groups accumulate onto
                        # the same psum but keep dependency-coalescing scoped
                        # so the chain can start before late weight chunks land
                        nc.tensor.matmul(
                            psq[t2][:], lhsT, wqkv_sb[:, ct, 0:512],
                            start=(ct == 0), stop=(ct % 4 == 3),
                            skip_group_check=True,
                        )
                        nc.tensor.matmul(
                            pskv[t2][:], lhsT, wqkv_sb[:, ct, 512:768],
                            start=(ct == 0), stop=(ct % 4 == 3),
                            skip_group_check=True,
                        )
                if ch == 0:
                    load_consts()
                # downcast psum -> bf16 on Act. Emission order matters: the
                # k/v copies free the pskv psum bufs the next chunk's matmuls
                # need, so they go first; the deferred transposes' psum->qT/kT
                # copies queue behind them on Act (no urgency — their readers
                # are the attention phase).
                qkv_bfs = []
                for t2 in range(2):
                    tt = ch * 2 + t2
                    qkv_bf = qbfp.tile([128, 640], bf16, tag="qbf")
                    qkv_bfs.append(qkv_bf)
                    nc.scalar.copy(qkv_bf[:, 512:640], pskv[t2][:, 0:128])
                    nc.scalar.copy(v_sb[:, tt, :], pskv[t2][:, 128:256])
                for t2 in range(2):
                    nc.scalar.copy(qkv_bfs[t2][:, 0:512], psq[t2][:])
                # PE-transpose the PREVIOUS chunk's rope output while this
                # chunk's rope runs on DVE (keeps the PE dense)
                for args in pending_tr:
                    emit_tr(*args)
                pending_tr = []
                # rope in bf16 (DVE at 2x)
                for t2 in range(2):
                    tt = ch * 2 + t2
                    qkv_bf = qkv_bfs[t2]
                    qr = qrp.tile([128, 512], bf16, tag="qr")
                    kr = krp.tile([128, 128], bf16, tag="kr")
                    cosb = bcast_mid(fcos_sb[:, tt, :], 4)
                    sinb = bcast_mid(fsin_sb[:, tt, :], 4)
                    qsrc = qkv_bf[:, 0:512].rearrange(
                        "p (h two j) -> p h two j", h=4, two=2
                    )
                    qdst = qr[:].rearrange(
                        "p (h two j) -> p h two j", h=4, two=2
                    )
                    te4, to4 = qsrc[:, :, 0, :], qsrc[:, :, 1, :]
                    a1 = rtp.tile([128, 4, 64], bf16, tag="rt")
                    a2 = rtp.tile([128, 4, 64], bf16, tag="rt")
                    nc.vector.tensor_mul(a1[:], te4, cosb)
                    nc.vector.tensor_mul(a2[:], to4, sinb)
                    nc.vector.tensor_sub(qdst[:, :, 0, :], a1[:], a2[:])
                    b1 = rtp.tile([128, 4, 64], bf16, tag="rt")
                    b2 = rtp.tile([128, 4, 64], bf16, tag="rt")
                    nc.vector.tensor_mul(b1[:], te4, sinb)
                    nc.vector.tensor_mul(b2[:], to4, cosb)
                    nc.vector.tensor_add(qdst[:, :, 1, :], b1[:], b2[:])
                    # K rope
                    kte, kto = qkv_bf[:, 512:576], qkv_bf[:, 576:640]
                    cos1 = fcos_sb[:, tt, :]
                    sin1 = fsin_sb[:, tt, :]
                    c1 = rtp.tile([128, 64], bf16, tag="rtk")
                    c2 = rtp.tile([128, 64], bf16, tag="rtk")
                    nc.vector.tensor_mul(c1[:], kte, cos1)
                    nc.vector.tensor_mul(c2[:], kto, sin1)
                    nc.vector.tensor_sub(kr[:, 0:64], c1[:], c2[:])
                    d1 = rtp.tile([128, 64], bf16, tag="rtk")
                    d2 = rtp.tile([128, 64], bf16, tag="rtk")
                    nc.vector.tensor_mul(d1[:], kte, sin1)
                    nc.vector.tensor_mul(d2[:], kto, cos1)
                    nc.vector.tensor_add(kr[:, 64:128], d1[:], d2[:])
                    pending_tr.append((tt, qr, kr))
                scope.__exit__(None, None, None)
            for args in pending_tr:
                emit_tr(*args)

            # ================= Phase 2+3: attention + output projection ======
            # Per t-chunk: scores for a head, then den/AV. exp on Act (544ns
            # per tile) is ~2.3x slower than a scores matmul, so the PE would
            # outrun exp and stall on the psum rotation. The previous chunk's
            # WO matmuls are exp-independent: they're interleaved into the
            # sc0/sc1 stretch as PE filler while Act drains the exp backlog.
            # All psum->sbuf copies in this phase go on DVE to keep Act free
            # for exp.
            def make_wo_pieces(tc_i):
                pieces = []
                for t2 in range(4):
                    gt = tc_i * 4 + t2
                    for cc2 in range(2):
                        def piece(gt=gt, cc2=cc2, k=t2 * 2 + cc2):
                            osb = opool.tile([128, 1024], bf16, tag="osb",
                                             name="osb")
                            for half in range(2):
                                cc = cc2 * 2 + half
                                psw = pc.tile([128, 512], f32, tag="pc",
                                              name="psw")
                                for h in range(G):
                                    nc.tensor.matmul(
                                        psw[:],
                                        outT_sb[:, h,
                                                gt * 128:(gt + 1) * 128],
                                        wo_sb[:, h,
                                              cc * 512:(cc + 1) * 512],
                                        start=(h == 0), stop=(h == G - 1),
                                    )
                                nc.vector.tensor_copy(
                                    osb[:, half * 512:(half + 1) * 512],
                                    psw[:])
                            store_eng = nc.sync if k % 2 else nc.gpsimd
                            store_eng.dma_start(
                                out[gt * 128:(gt + 1) * 128,
                                    cc2 * 1024:(cc2 + 1) * 1024],
                                osb[:],
                            )
                        pieces.append(piece)
                return pieces

            pending_wo = []
            for tc_i in range(NTC):
                scope = nc.named_scope(f"attn_tc{tc_i}")
                scope.__enter__()
                t0 = tc_i * 512
                n_s = 4 * (tc_i + 1)
                offs = [128 * (si - 4 * tc_i) if si >= 4 * tc_i else 0
                        for si in range(n_s)]
                # natural si order: si=0 always has off=0 (so the first
                # matmul of each psum accumulation group resets the full
                # 512-col range), and the diagonal tiles — whose exps are
                # emitted last — land at the END of the den/AV chains,
                # giving the Act exp pipeline maximum slack
                order = list(range(n_s))
                expTs = {}

                def emit_scores(h, sis, tc_i=tc_i, t0=t0, offs=offs,
                                expTs=expTs):
                    if h not in expTs:
                        expT_t = expp.tile([128, TT, 512], bf16,
                                           tag="expT", name="expT")
                        e8_t = e8p.tile([128, 6, 2, 512], fp8, tag="e8",
                                        name="e8_t")
                        expTs[h] = (expT_t, e8_t)
                    expT, e8 = expTs[h]
                    for si in sis:
                        off = offs[si]
                        ps = pa.tile([128, 512], f32, tag="pa", name="ps")
                        nc.tensor.matmul(
                            ps[:, off:512],
                            kT_sb[:, si * 128:(si + 1) * 128],
                            qT_sb[:, h, t0 + off:t0 + 512],
                            start=True, stop=True,
                        )
                        if si >= 4 * tc_i:
                            nc.vector.tensor_add(
                                ps[:, off:off + 128],
                                ps[:, off:off + 128], tri_sb[:],
                            )
                        nc.scalar.activation(
                            expT[:, si, off:512], ps[:, off:512],
                            mybir.ActivationFunctionType.Exp, scale=SCALE,
                        )
                        if si < 4 * tc_i:
                            # fp8 copies feeding the DoubleRow den pass,
                            # alternated DVE/Act so a pair is never gated on
                            # a single backlogged queue
                            if si % 2:
                                nc.scalar.copy(
                                    e8[:, si // 2, si % 2, :],
                                    expT[:, si, 0:512],
                                )
                            else:
                                nc.vector.tensor_copy(
                                    e8[:, si // 2, si % 2, :],
                                    expT[:, si, 0:512],
                                )

                def emit_da(h, tc_i=tc_i, t0=t0, n_s=n_s, offs=offs,
                            order=order, expTs=expTs):
                    expT, e8 = expTs.pop(h)
                    # full (off-diagonal) s-tile pairs via fp8 DoubleRow
                    # (K=256 per instruction, half the PE cost); row 0 of the
                    # 64-row output carries the sum, rows 1-63 are padding to
                    # satisfy the dual-fp8 weight-load width restriction.
                    # Diagonal tiles accumulate onto row 0 in bf16.
                    psd = pb.tile([64, 512], f32, tag="pb", name="psd")
                    npair = 2 * tc_i
                    nden = npair + (n_s - 4 * tc_i)
                    for i in range(npair):
                        nc.tensor.matmul(
                            psd[:, 0:512],
                            ones8_sb[:].rearrange("p (two m) -> p two m",
                                                  two=2),
                            e8[:, i, :, :],
                            perf_mode=mybir.MatmulPerfMode.DoubleRow,
                            start=(i == 0), stop=(i == nden - 1),
                        )
                    for j, si in enumerate(range(4 * tc_i, n_s)):
                        i = npair + j
                        off = offs[si]
                        nc.tensor.matmul(
                            psd[0:1, off:512], ones_sb[:],
                            expT[:, si, off:512],
                            start=(i == 0), stop=(i == nden - 1),
                        )
                    den_r = denp.tile([1, 512], f32, tag="denr",
                                      name="den_r")
                    nc.vector.reciprocal_approx_fast(den_r[:], psd[0:1, :])
                    bc = bcp.tile([128, 512], f32, tag="bc", name="bc")
                    nc.gpsimd.partition_broadcast(bc[:], den_r[:])
                    pso = pa.tile([128, 512], f32, tag="pa", name="pso")
                    for i, si in enumerate(order):
                        off = offs[si]
                        nc.tensor.matmul(
                            pso[:, off:512], v_sb[:, si, :],
                            expT[:, si, off:512],
                            start=(i == 0), stop=(i == n_s - 1),
                        )
                    nc.vector.tensor_mul(
                        outT_sb[:, h, t0:t0 + 512], pso[:], bc[:]
                    )

                # sc0/sc1 in si-quads with the previous chunk's WO pieces as
                # filler, then the 1-deep head pipeline for the rest
                quads = [(h, list(range(q, min(q + 4, n_s))))
                         for h in (0, 1) for q in range(0, n_s, 4)]
                npw = max(0, len(pending_wo) - 2)  # save 2 for da0/da1 slots
                k = 0
                for qi, (h, sis) in enumerate(quads):
                    emit_scores(h, sis)
                    owed = ((qi + 1) * npw) // len(quads)
                    while k < owed:
                        pending_wo[k]()
                        k += 1
                while k < npw:
                    pending_wo[k]()
                    k += 1
                emit_da(0)
                if k < len(pending_wo):
                    pending_wo[k]()
                    k += 1
                emit_scores(2, range(n_s))
                emit_da(1)
                if k < len(pending_wo):
                    pending_wo[k]()
                    k += 1
                emit_scores(3, range(n_s))
                emit_da(2)
                emit_da(3)
                pending_wo = make_wo_pieces(tc_i)
                scope.__exit__(None, None, None)
            scope = nc.named_scope("wo_tc3")
            scope.__enter__()
            for p in pending_wo:
                p()
            scope.__exit__(None, None, None)

    nc.finalize()
    return nc


def _prep_host(x, freqs_cos, freqs_sin, wq, wk, wv, wo):
    """Build per-core input maps."""
    x = np.asarray(x, dtype=np.float32)
    freqs_cos = np.asarray(freqs_cos, dtype=np.float32)
    freqs_sin = np.asarray(freqs_sin, dtype=np.float32)
    wq = np.asarray(wq, dtype=np.float32)
    wk = np.asarray(wk, dtype=np.float32)
    wv = np.asarray(wv, dtype=np.float32)
    wo = np.asarray(wo, dtype=np.float32)

    perm = np.concatenate([np.arange(0, HD, 2), np.arange(1, HD, 2)])
    # xT pre-tiled: [p, ch, cg, ci, t'] so each (ch, cg) load is contiguous
    xTs = []
    for b in range(B):
        A = np.ascontiguousarray(x[b].T)           # [C, T]
        A = A.reshape(4, 4, 128, 8, 256)           # [cg, ci, p, ch, t']
        A = A.transpose(2, 3, 0, 1, 4)             # [p, ch, cg, ci, t']
        xTs.append(np.ascontiguousarray(A.reshape(128, -1).astype(BF16)))
    cident = np.eye(128, dtype=BF16)
    cones = np.ones((128, 1), dtype=BF16)
    cones8 = np.ones((128, 128), dtype=ml_dtypes.float8_e4m3)
    ds, dt = np.meshgrid(np.arange(128), np.arange(128), indexing="ij")
    ctri = np.where(ds <= dt, 0.0, MASK_BIAS).astype(np.float32)

    in_maps = []
    for c in range(NCORES):
        b, kv = c // 4, c % 4
        cols = []
        for g in range(G):
            h = kv * G + g
            cols.append(wq[:, h * HD:(h + 1) * HD][:, perm])
        cols.append(wk[:, kv * HD:(kv + 1) * HD][:, perm])
        cols.append(wv[:, kv * HD:(kv + 1) * HD])
        wqkv_c = np.concatenate(cols, axis=1)              # [C, 768]
        wqkv_c = wqkv_c.reshape(CT, 128, 768).transpose(1, 0, 2)
        wqkv_c = np.ascontiguousarray(wqkv_c.reshape(128, -1).astype(BF16))
        wo_c = wo[kv * G * HD:(kv + 1) * G * HD, :]        # [512, C]
        wo_c = wo_c.reshape(G, 128, C).transpose(1, 0, 2)
        wo_c = np.ascontiguousarray(wo_c.reshape(128, -1).astype(BF16))
        fc = np.ascontiguousarray(
            freqs_cos.reshape(TT, 128, 64).transpose(1, 0, 2)
            .reshape(128, -1).astype(BF16))
        fs = np.ascontiguousarray(
            freqs_sin.reshape(TT, 128, 64).transpose(1, 0, 2)
            .reshape(128, -1).astype(BF16))
        in_maps.append({
            "xT": xTs[b],
            "wqkv": wqkv_c,
            "wo": wo_c,
            "fcos": fc,
            "fsin": fs,
            "cident": cident,
            "cones": cones,
            "cones8": cones8,
            "ctri": ctri,
        })
    return in_maps


def _install_ntff_hook_shim():
    """bass_utils trace=True needs antenv.axon_hooks, absent in this image.
    Provide it in sys.modules and register the ctypes NTFF hook."""
    import types

    if "antenv.axon_hooks" in sys.modules:
        return
    mod = types.ModuleType("antenv.axon_hooks")
    mod._hook = None
    mod.set_axon_ntff_profile_hook = lambda h: setattr(mod, "_hook", h)
    mod.get_axon_ntff_profile_hook = lambda: mod._hook
    sys.modules["antenv.axon_hooks"] = mod
    try:
        from trn_agent_boot.trn_boot import _ntff_profile_via_ctypes

        mod._hook = _ntff_profile_via_ctypes("/opt/axon/libaxon_pjrt.so")
    except Exception:
        pass


def kernel(x, freqs_cos, freqs_sin, wq, wk, wv, wo, trace=False):
    global LAST_RESULTS
    from concourse.bass_utils import run_bass_kernel_spmd

    if trace:
        _install_ntff_hook_shim()

    if "nc" not in _CACHE:
        _CACHE["nc"] = _build()
    nc = _CACHE["nc"]

    in_maps = _prep_host(x, freqs_cos, freqs_sin, wq, wk, wv, wo)
    res = run_bass_kernel_spmd(nc, in_maps, core_ids=list(range(NCORES)),
                               trace=trace)
    LAST_RESULTS = res
    out = np.zeros((B, T, C), dtype=np.float32)
    for c in range(NCORES):
        out[c // 4] += res.results[c]["out"].astype(np.float32)
    return out
